# revision 55
# baseline (speedup 1.0000x reference)
"""Multi-head attention (B=8, S=2048, E=1024, H=16, D=64) on 8 TRN2 NeuronCores.

Sharding: data parallel over batch — core b computes batch b end to end.

Per-core device kernel:
  QK^T        fp8e4m3 DoubleRow matmuls at 0.5 cycles/column: q and k are
              each split hi+lo (q ~= qh+ql, both fp8); the 256-slot double-row
              contraction computes (qh+ql)·(kh+kl) exactly — all four cross
              terms — so accuracy is fp16-grade while QK runs 2x faster.
              K is stored once ([kh;kl] stacked on partitions) and fed with a
              stride-0 broadcast AP on the double-row axis; Q stores the two
              slots explicitly ([qh;ql] and [ql;qh]).
  expS        one op per j-tile covering BOTH heads of the pair: the two
              heads' score banks are adjacent in PSUM, so a single [128,1024]
              activation (ACT exact exp) or tensor_scalar (DVE Schraudolph
              fast-exp) amortizes the fixed PSUM/SBUF access penalty.
  AV          exp tile stationary, [v|1] moving (65 cols); one PSUM
              accumulation group per (pair, head) covering all 4 i-subtiles.
  norm        one reciprocal [128,4] + one broadcast tensor_tensor per
              (pair, head) normalizes all 4 subtiles in a single DVE op.
  transpose   XBAR DMA transpose (SBUF->SBUF), cat2 -> catT in one DMA per
              pair; no PE or DVE involvement.
  proj        catT chunks x W columns on PE; bias-add + PSUM->SBUF copy fused
              in one DVE tensor_tensor; DMA to DRAM.
"""

import sys

if "/opt/trn_rl_repo" not in sys.path:
    sys.path.insert(0, "/opt/trn_rl_repo")

from contextlib import ExitStack

import numpy as np

B, S, E, H, D = 8, 2048, 1024, 16, 64
P = 128            # partitions
IC = 512           # query-position chunk per inner loop
NI = S // IC       # 4 i-chunks
NJ = S // P        # 16 key-position tiles
NK = E // P        # 8 contraction chunks in the output projection
NO = 2             # e_out chunks of 512
NP_ = H // 2       # 8 head pairs
NSUB = IC // P     # 4 i-subtiles of 128 per i-chunk
SCALE = 1.0 / 8.0  # 1/sqrt(D)

# Schraudolph fast-exp: exp(s/8) ~= bitcast_fp16(u16(s * A + B)).
A_SCH = 1024.0 * 1.4426950408889634 * SCALE
B_SCH = 15360.0 - 62.0
# engine for the exp unit of j-tile gi (both heads at once):
# A = exact ACT exp, D = DVE Schraudolph fast-exp
# interleaved so DVE never idles behind an ACT run at the pair wrap; the two
# AA runs (gi 2-3, 9-10) are where DVE's norms and output copy are scheduled
EXP_ENGINE = "ADAADADADAADADAD"  # 9 ACT / 7 DVE
# AV blocks (0..7 = hb-major, isub-minor) of pair t-1 paced after these QK
# slots of pair t
AV_PACING = {2: 0, 4: 1, 6: 2, 8: 3, 10: 4, 12: 5, 14: 6, 15: 7}

_NC_CACHE = {}


def _build_nc():
    import concourse.mybir as mybir
    import concourse.tile as tile
    from concourse import bacc

    f32 = mybir.dt.float32
    f16 = mybir.dt.float16
    f8 = mybir.dt.float8e4
    u16 = mybir.dt.uint16
    Exp = mybir.ActivationFunctionType.Exp
    Copy = mybir.ActivationFunctionType.Copy
    mult = mybir.AluOpType.mult
    add = mybir.AluOpType.add
    DR = mybir.MatmulPerfMode.DoubleRow

    nc = bacc.Bacc(
        "TRN2",
        target_bir_lowering=False,
        debug=False,
        enable_asserts=False,
        num_devices=8,
    )

    q8_d = nc.dram_tensor("q8", [P, H, 2, S], f8, kind="ExternalInput")
    k8_d = nc.dram_tensor("k8", [P, H, S], f8, kind="ExternalInput")
    # [p, hp, jt, 130]: [v_even(64) | 1 | v_odd(64) | 1]; ones feed softmax sums
    vt_d = nc.dram_tensor("vt", [P, NP_, NJ, 130], f16, kind="ExternalInput")
    wt_d = nc.dram_tensor("wt", [E, E], f16, kind="ExternalInput")
    bi_d = nc.dram_tensor("bias", [1, E], f16, kind="ExternalInput")
    on_d = nc.dram_tensor("ones1", [1, P], f16, kind="ExternalInput")
    out_d = nc.dram_tensor("out", [S, E], f32, kind="ExternalOutput")

    with tile.TileContext(nc) as tc, ExitStack() as ctx:
        const = ctx.enter_context(tc.tile_pool(name="const", bufs=1))
        qpool = ctx.enter_context(tc.tile_pool(name="qpool", bufs=2))
        epool = ctx.enter_context(tc.tile_pool(name="epool", bufs=2))
        cpool = ctx.enter_context(tc.tile_pool(name="cpool", bufs=3))
        rpool = ctx.enter_context(tc.tile_pool(name="rpool", bufs=4))
        opool = ctx.enter_context(tc.tile_pool(name="opool", bufs=2))
        # 3-slot score ring (2 banks each) + 2 acc banks = 8 PSUM banks; the
        # projection accumulator borrows a score-ring slot (same tag) for the
        # ~2 gi slots a proj task is in flight.
        spool = ctx.enter_context(tc.tile_pool(name="spool", bufs=3, space="PSUM"))
        apool = ctx.enter_context(tc.tile_pool(name="apool", bufs=2, space="PSUM"))

        # --- persistent tiles ------------------------------------------------
        kt_all = const.tile([P, H, S], f8)
        vt_all = const.tile([P, NP_, NJ, 130], f16)
        wt_all = const.tile([P, NK, E], f16)
        catT = const.tile([P, NK, 2, IC], f16)  # ring: slot ii % 2
        bias16 = const.tile([1, E], f16)
        ones1 = const.tile([1, P], f16)

        qt_tiles = {}

        def load_qt(ii):
            qt_tiles[ii] = qpool.tile([P, H, 2, IC], f8, tag="qt",
                                      name=f"qt_{ii}")

        def load_qt_pair(ii, p):
            isl = slice(ii * IC, (ii + 1) * IC)
            hsl = slice(2 * p, 2 * p + 2)
            nc.sync.dma_start(
                qt_tiles[ii][:, hsl, :, :], q8_d.ap()[:, hsl, :, isl]
            )

        # all input DMAs go on the SP ring in need-order so the ACT/DVE
        # sequencers stay free to decode exps from the first microsecond;
        # the first two j-tiles of pair 0's K go first so QK starts at ~1us
        nc.sync.dma_start(kt_all[:, 0:2, 0:256], k8_d.ap()[:, 0:2, 0:256])
        load_qt(0)
        load_qt_pair(0, 0)
        nc.sync.dma_start(kt_all[:, 0:2, 256:], k8_d.ap()[:, 0:2, 256:])
        nc.sync.dma_start(vt_all[:, 0, :, :], vt_d.ap()[:, 0, :, :])
        for p in range(1, NP_):
            nc.sync.dma_start(kt_all[:, 2 * p : 2 * p + 2, :],
                              k8_d.ap()[:, 2 * p : 2 * p + 2, :])
            load_qt_pair(0, p)
            nc.sync.dma_start(vt_all[:, p, :, :], vt_d.ap()[:, p, :, :])
        wt_r = wt_d.ap().rearrange("(ko ki) o -> ki ko o", ki=P)
        nc.sync.dma_start(bias16[:], bi_d.ap())
        nc.sync.dma_start(ones1[:], on_d.ap())

        # --- emission helpers ------------------------------------------------
        ex_tiles = {}     # t -> ex tile [P, NJ, 2, IC]
        acc_tiles = {}    # (t, hbi) -> acc [P, NSUB, 65]
        cat_tiles = {}    # t -> cat2 [P, NSUB, P]
        rc_tiles = {}     # (t, hbi) -> rc [P, NSUB]
        norm_queue = []   # [(t, hbi)] delayed merged norms
        pending_proj = []
        proj_half = [None]

        def emit_qk(t, p, gi):
            qt_ii = qt_tiles[t // NP_]
            sc = spool.tile([P, 2, IC], f32, tag="sc", name=f"sc_{t}_{gi}")
            jsl = slice(gi * P, (gi + 1) * P)
            for hbi in (0, 1):
                h = 2 * p + hbi
                lhsT = kt_all[:, h, jsl].unsqueeze(1).broadcast_to([P, 2, P])
                for half in (0, 1):
                    isl = slice(half * 256, (half + 1) * 256)
                    nc.tensor.matmul(
                        sc[:, hbi, isl],
                        lhsT,
                        qt_ii[:, h, :, isl],
                        start=True, stop=True,
                        perf_mode=DR,
                    )
            return sc

        def emit_exp(t, gi, sc):
            if t not in ex_tiles:
                ex_tiles[t] = epool.tile([P, NJ, 2, IC], f16, tag="ex",
                                         name=f"ex_{t}")
            ex = ex_tiles[t]
            dst = ex[:, gi, :, :]
            if EXP_ENGINE[gi] == "A":
                nc.scalar.activation(dst, sc[:], Exp, scale=SCALE)
            else:
                nc.vector.tensor_scalar(
                    dst.bitcast(u16), sc[:],
                    float(A_SCH), float(B_SCH), mult, add,
                )

        av_count = {}  # (t, hbi) -> matmuls emitted into acc (0..64)

        def emit_av_half(t, hbi, isub, jhalf):
            p = t % NP_
            ex = ex_tiles[t]
            vb = 65 * hbi
            key = (t, hbi)
            if key not in acc_tiles:
                acc_tiles[key] = apool.tile([P, NSUB, 65], f32, tag="acc",
                                            name=f"acc_{t}_{hbi}")
                av_count[key] = 0
            acc = acc_tiles[key]
            n = av_count[key]
            for jt in range(jhalf * 8, jhalf * 8 + 8):
                nc.tensor.matmul(
                    acc[:, isub, :],
                    ex[:, jt, hbi, isub * P : (isub + 1) * P],
                    vt_all[:, p, jt, vb : vb + 65],
                    start=(n == 0),
                    stop=(n == 2 * NSUB * 8 - 1),
                )
                n += 1
            av_count[key] = n
            if n == 2 * NSUB * 8:
                rc = rpool.tile([P, NSUB], f32, tag="rc")
                nc.vector.reciprocal(rc[:], acc[:, :, 64:65])
                rc_tiles[key] = rc
                norm_queue.append(key)
                av_count.pop(key)
                if hbi == 1:
                    ex_tiles.pop(t)

        def emit_av_block(t, b):
            # b: 0..7, hb-major: hbi = b // NSUB, isub = b % NSUB
            hbi, isub = b // NSUB, b % NSUB
            emit_av_half(t, hbi, isub, 0)
            emit_av_half(t, hbi, isub, 1)

        def flush_norms():
            while norm_queue:
                t, hbi = norm_queue.pop(0)
                acc = acc_tiles.pop((t, hbi))
                rc = rc_tiles.pop((t, hbi))
                if t not in cat_tiles:
                    cat_tiles[t] = cpool.tile([P, NSUB, P], f16, tag="cat",
                                              name=f"cat_{t}")
                cat2 = cat_tiles[t]
                nc.vector.tensor_tensor(
                    cat2[:, :, 64 * hbi : 64 * hbi + 64],
                    acc[:, :, 0:64],
                    rc[:].unsqueeze(2).broadcast_to([P, NSUB, 64]),
                    mult,
                )

        def emit_pair_end(t):
            # cat2 [i, (isub, e)] -> catT [e, (isub, i)] via XBAR DMA transpose
            p = t % NP_
            ii = t // NP_
            cat2 = cat_tiles.pop(t)
            dst = catT[:, p, ii % 2, :].rearrange("e (s i) -> e s i", s=NSUB)
            nc.sync.dma_start_transpose(dst, cat2[:, :, :])

        def emit_proj_part1(ii, it, o):
            i0 = it * P
            osl = slice(o * 512, (o + 1) * 512)
            pp = apool.tile([P, 512], f32, tag="acc", name="pp")
            for k in range(6):
                nc.tensor.matmul(
                    pp[:], catT[:, k, ii % 2, i0 : i0 + P], wt_all[:, k, osl],
                    start=(k == 0), stop=False,
                )
            proj_half[0] = (pp, ii, it, o)

        out_flip = [0]

        def emit_proj_part2():
            pp, ii, it, o = proj_half[0]
            proj_half[0] = None
            i0 = it * P
            osl = slice(o * 512, (o + 1) * 512)
            for k in range(6, NK):
                nc.tensor.matmul(
                    pp[:], catT[:, k, ii % 2, i0 : i0 + P], wt_all[:, k, osl],
                    start=False, stop=False,
                )
            nc.tensor.matmul(pp[:], ones1[:], bias16[:, osl],
                             start=False, stop=True)
            ob = opool.tile([P, 512], f32)
            out_flip[0] ^= 1
            if out_flip[0]:
                nc.vector.tensor_copy(ob[:], pp[:])
            else:
                nc.scalar.activation(ob[:], pp[:], Copy)
            od = ii * IC + it * P
            nc.sync.dma_start(out_d.ap()[od : od + P, osl], ob[:])

        # --- pipelined emission ----------------------------------------------
        pairs = [(ii, p) for ii in range(NI) for p in range(NP_)]
        exp_queue = []  # [(t, gi, sc)] — QK leads exp by two slots

        for t, (ii, p) in enumerate(pairs):
            prev = t - 1 if t >= 1 else None
            prev2 = t - 2 if t >= 2 else None
            if ii + 1 < NI:
                if p == 0:
                    load_qt(ii + 1)
                load_qt_pair(ii + 1, p)
            if t < NK:
                # W chunk loads spread over chunk 0, after each pair's q slice
                nc.sync.dma_start(wt_all[:, t, :], wt_r[:, t, :])
            for gi in range(NJ):
                if len(exp_queue) == 1:
                    emit_exp(*exp_queue.pop(0))
                if gi == 4 and proj_half[0] is not None:
                    emit_proj_part2()
                if prev is not None and gi in AV_PACING:
                    emit_av_block(prev, AV_PACING[gi])
                if gi in (4, 9):
                    flush_norms()
                sc = emit_qk(t, p, gi)
                exp_queue.append((t, gi, sc))
            # pair boundary: transpose of pair t-2, half of one proj task
            if prev2 is not None:
                emit_pair_end(prev2)
                if pending_proj:
                    emit_proj_part1(*pending_proj.pop(0))
            # i-chunk ii's catT complete once pair (ii+1, 0) ends -> queue proj
            if prev2 is not None and prev2 % NP_ == NP_ - 1:
                cii = prev2 // NP_
                pending_proj.extend(
                    (cii, it, o) for it in range(NSUB) for o in range(NO)
                )

        # --- drain ------------------------------------------------------------
        last = len(pairs) - 1
        while exp_queue:
            emit_exp(*exp_queue.pop(0))
        if proj_half[0] is not None:
            emit_proj_part2()
        for b in range(8):
            emit_av_block(last, b)
            flush_norms()
        emit_pair_end(last - 1)
        emit_pair_end(last)
        pending_proj.extend(
            (NI - 1, it, o) for it in range(NSUB) for o in range(NO)
        )
        while pending_proj:
            emit_proj_part1(*pending_proj.pop(0))
            emit_proj_part2()

    nc.compile()
    return nc


def get_nc():
    if "nc" not in _NC_CACHE:
        _NC_CACHE["nc"] = _build_nc()
    return _NC_CACHE["nc"]


def make_in_maps(values, keys, queries, W_out, b_out):
    import ml_dtypes

    f16 = np.float16
    f8 = ml_dtypes.float8_e4m3

    def hilo(x):
        # x: [B, H, D, S] fp32 -> (hi, lo) fp8 with x ~= hi + lo
        hi = x.astype(f8)
        lo = (x - hi.astype(np.float32)).astype(f8)
        return hi, lo

    q32 = (
        np.asarray(queries, dtype=np.float32)
        .reshape(B, S, H, D)
        .transpose(0, 2, 3, 1)
    )  # [B, H, D, S]
    k32 = (
        np.asarray(keys, dtype=np.float32)
        .reshape(B, S, H, D)
        .transpose(0, 2, 3, 1)
    )
    qh, ql = hilo(q32)
    kh, kl = hilo(k32)
    # q8: [B, P, H, 2, S]; slot0 rows = [qh; ql], slot1 rows = [ql; qh]
    q8 = np.empty((B, P, H, 2, S), dtype=f8)
    q8[:, 0:D, :, 0] = qh.transpose(0, 2, 1, 3)
    q8[:, D:P, :, 0] = ql.transpose(0, 2, 1, 3)
    q8[:, 0:D, :, 1] = ql.transpose(0, 2, 1, 3)
    q8[:, D:P, :, 1] = qh.transpose(0, 2, 1, 3)
    # k8: [B, P, H, S]; rows = [kh; kl]
    k8 = np.empty((B, P, H, S), dtype=f8)
    k8[:, 0:D] = kh.transpose(0, 2, 1, 3)
    k8[:, D:P] = kl.transpose(0, 2, 1, 3)

    v = np.asarray(values, dtype=np.float32).reshape(B, S, H, D)
    vt = np.empty((B, S, H, D + 1), dtype=f16)
    vt[..., :D] = v.astype(f16)
    vt[..., D] = np.float16(1.0)
    # [B, S, H, 65] -> [B, jt, p, hp, 130] -> [B, p, hp, jt, 130]
    vt = np.ascontiguousarray(
        vt.reshape(B, NJ, P, NP_, 130).transpose(0, 2, 3, 1, 4)
    )
    wt = np.ascontiguousarray(np.asarray(W_out, dtype=np.float32).T).astype(f16)
    bias = np.asarray(b_out, dtype=np.float32).reshape(1, E).astype(f16)
    ones1 = np.ones((1, P), dtype=f16)
    return [
        {"q8": q8[b], "k8": k8[b], "vt": vt[b], "wt": wt, "bias": bias,
         "ones1": ones1}
        for b in range(B)
    ]


def kernel(values, keys, queries, W_out, b_out):
    from concourse.bass_utils import run_bass_kernel_spmd

    nc = get_nc()
    in_maps = make_in_maps(values, keys, queries, W_out, b_out)
    res = run_bass_kernel_spmd(nc, in_maps, core_ids=list(range(8)))
    out = np.stack([res.results[b]["out"] for b in range(B)], axis=0)
    return np.ascontiguousarray(out.astype(np.float32))
